# revision 16
# baseline (speedup 1.0000x reference)
"""FFT causal long-conv (H3/Hyena fftconv) as a blocked-Toeplitz matmul kernel
for 8 Trainium2 NeuronCores, with one level of block-Karatsuba.

Math: y[b,d,l] = sum_{t<=l} filter[d,t] * x[b,d,l-t]  (causal conv, L taps).

Direct form: with 128-wide blocks (J=L/128=32), y_i = sum_{k<=i} T_k x_{i-k},
T_k[a,c] = f[128k + a - c] -- 528 block-matmuls/channel (8448 PE stream cols),
which saturates the PE at ~1 col/cycle.

Karatsuba level 1 (halves split at H=L/2, quarters at Q=L/4):
  y_low  = (f0 * x0)_low                                      (A = F16)
  y_high = (f0 * x0)_high + G(f0, x0+x1)_low + G(f1-f0, x0)_low
  F16 (full conv f0*x0) via three quarter-size FULL convs:
    A1 = f00*x00,  A2 = f01*x01,  AD = (f01-f00)*(x01-x00)
    F16 = A1 <<0  +  A2 <<Q2  +  (A1 + A2 - AD) <<Q      (shifts in samples)
This cuts PE stream columns to ~7800/channel (0.92x).  The e3m4 weight tiles
dedupe to 42 tile-sets/channel: A1's lag-0..6 and A2's lag-1..7 tiles are
exactly G1's lag-0..6 / 9..15 tiles (same quantized f0 Toeplitz windows); only
the quarter-boundary variants differ.  AD ships negated so RD accumulates -AD
and every assembly step is an add.  Max rel err ~1.9e-2 (numpy-exact sim of
the full scheme; gate is 2e-2; the HW matched the same sim to 1e-6 on the
direct kernel).

Weights are fp8 E3M4 (filter pre-scaled by 64, x carries 1/64): the PE streams
fp8e3 lhsT at the same 1 col/cycle as fp16 but weight DMA bytes halve, keeping
all 16 SDMA engines off the critical path (~132MB/core vs PE ~430us).

Sharding: channels D=1024 split 128 per core; all B=16 batches stay on-core.
"""

import numpy as np
import ml_dtypes


B, D, L = 16, 1024, 4096
NCORES = 8
DC = D // NCORES  # channels per core
C = 128           # time-block size == PE contraction dim
J = L // C        # 32 time blocks
JH = J // 2       # 16 blocks per half
JQ = J // 4       # 8 blocks per quarter
N = J * B         # 512 = full free dim (j-block outer, batch inner)
NH = JH * B       # 256
NQ = JQ * B       # 128
NF = 2 * NQ       # 256 = full-conv output cols of a quarter piece (16 blocks)
GROUP = 4         # channels per DMA batch
NT = 45           # weight tile-sets per channel (see layout below)

F16 = np.float16
F8 = ml_dtypes.float8_e3m4
FSCALE = 64.0     # filter pre-scale into e3m4's sweet spot; x carries 1/64

# A full conv of two 8-block (1024-sample) pieces spans 16 block-lags and
# needs NINE lag tiles (0..8; lag 8 is the triangular boundary tile).
# ft tile layout along axis 2:
#   [0:16]   G1: Toeplitz of f0 (lags 0..15); lags 0..6 double as A1's k=0..6,
#            lags 9..15 double as A2's k=1..7
#   [16:18]  A1 lag 7, 8 boundary variants (f00-only window)
#   [18:20]  A2 lag 0, 8 boundary variants (f01-only window)
#   [20:36]  G2: Toeplitz of f1-f0 (lags 0..15)
#   [36:45]  AD: Toeplitz of f00-f01 (= negated f01-f00), lags 0..8
A1_TILES = [0, 1, 2, 3, 4, 5, 6, 16, 17]
A2_TILES = [18, 9, 10, 11, 12, 13, 14, 15, 19]
AD_TILES = [36 + k for k in range(JQ + 1)]
G2_BASE = 20

_CACHE = {}


def _build_nc():
    if "nc" in _CACHE:
        return _CACHE["nc"]

    import concourse.bacc as bacc
    import concourse.tile as tile
    import concourse.mybir as mybir

    nc = bacc.Bacc("TRN2", target_bir_lowering=False, debug=False, num_devices=NCORES)

    # xt[c, d, n]   n = j*B + b   (input, time-within-block on partitions)
    # zs[c, d, nh]  zsum = x0 + x1 (block-aligned half sum, 16 blocks)
    # xd[c, d, nq]  xdiff = x01 - x00 (quarter diff, 8 blocks)
    # ft[c, d, t, a] 42 PE-ready lhsT tile-sets (layout above)
    # yt[a, d, n]   output
    xt = nc.dram_tensor("xt", [C, DC, N], mybir.dt.float16, kind="ExternalInput")
    zs = nc.dram_tensor("zs", [C, DC, NH], mybir.dt.float16, kind="ExternalInput")
    xd = nc.dram_tensor("xd", [C, DC, NQ], mybir.dt.float16, kind="ExternalInput")
    ft = nc.dram_tensor("ft", [C, DC, NT, C], mybir.dt.float8e3, kind="ExternalInput")
    yt = nc.dram_tensor("yt", [C, DC, N], mybir.dt.float16, kind="ExternalOutput")

    with tile.TileContext(nc) as tc:
        with (
            tc.tile_pool(name="wpool", bufs=3) as wpool,
            tc.tile_pool(name="xpool", bufs=3) as xpool,
            tc.tile_pool(name="ypool", bufs=3) as ypool,
            tc.tile_pool(name="spool", bufs=3) as spool,
            tc.tile_pool(name="pspool", bufs=3, space="PSUM") as pspool,
            tc.tile_pool(name="warmps", bufs=1, space="PSUM") as warmps,
        ):
            # Warm the PE (HAM un-throttle + weight-DMA cover): ~7us of
            # throwaway matmuls on a zeroed tile.  wz also serves as the
            # zero-weight operand for the PSUM-coverage "clear" matmuls.
            wz = wpool.tile([C, N], mybir.dt.float16, tag="warmz", bufs=1)
            nc.vector.memset(wz, 0.0)
            wps = warmps.tile([C, N], mybir.dt.float32)
            for _ in range(265):
                nc.tensor.matmul(wps[:, :64], wz[:, :C], wz[:, :64],
                                 start=True, stop=True)
            for g in range(DC // GROUP):
                sl = slice(g * GROUP, (g + 1) * GROUP)
                # Alternate the two HWDGE rings (SP + ACT) between groups.
                eng_a = nc.sync if g % 2 == 0 else nc.scalar
                eng_b = nc.scalar if g % 2 == 0 else nc.sync
                xg = xpool.tile([C, GROUP, N], mybir.dt.float16)
                eng_b.dma_start(out=xg, in_=xt[:, sl, :])
                zg = xpool.tile([C, GROUP, NH], mybir.dt.float16)
                eng_a.dma_start(out=zg, in_=zs[:, sl, :])
                dg = xpool.tile([C, GROUP, NQ], mybir.dt.float16)
                eng_b.dma_start(out=dg, in_=xd[:, sl, :])
                wt = wpool.tile([C, GROUP, NT, C], mybir.dt.float8e3)
                h = 3 * GROUP // 4
                eng_a.dma_start(out=wt[:, :h], in_=ft[:, sl.start:sl.start + h, :, :])
                eng_b.dma_start(out=wt[:, h:], in_=ft[:, sl.start + h:sl.stop, :, :])
                yg = ypool.tile([C, GROUP, N], mybir.dt.float16)
                for dd in range(GROUP):
                    w = wt[:, dd]
                    # Two PSUM banks per channel (full 2KB banks): R1|R2, RD|Rg.
                    bank_a = pspool.tile([C, N], mybir.dt.float32)
                    bank_b = pspool.tile([C, N], mybir.dt.float32)
                    r1 = bank_a[:, :NF]
                    r2 = bank_a[:, NF:]
                    rd = bank_b[:, :NF]
                    rg = bank_b[:, NF:]
                    # Rg = G(f0, zsum)_low + G(f1-f0, x0)_low.
                    # G1 k=0 covers all 256 cols with start=True.
                    nc.tensor.matmul(rg[:, :], w[:, 0], zg[:, dd, :],
                                     start=True, stop=False)
                    for k in range(1, JH):
                        nc.tensor.matmul(rg[:, k * B:], w[:, k],
                                         zg[:, dd, :NH - k * B],
                                         start=False, stop=False)
                    for k in range(JH):
                        nc.tensor.matmul(rg[:, k * B:], w[:, G2_BASE + k],
                                         xg[:, dd, :NH - k * B],
                                         start=False, stop=False)
                    # Quarter-size full convs: A1 -> R1, A2 -> R2, -AD -> RD.
                    # Lag-i matmul writes cols [i*B, i*B+NQ).  start=True
                    # poisons the WHOLE 2KB bank as pending-zero (per-element
                    # first-touch overwrites, later touches accumulate), so
                    # each bank gets exactly ONE start=True on its first MM:
                    # bank_a's at R1 lag 0 (R2 relies on the same poison),
                    # bank_b's was G1 k=0 above (RD relies on it).
                    for (rr, tiles, rhs, first) in (
                        (r1, A1_TILES, xg[:, dd, :NQ], True),
                        (r2, A2_TILES, xg[:, dd, NQ:NH], False),
                        (rd, AD_TILES, dg[:, dd, :], False),
                    ):
                        for i in range(JQ + 1):
                            nc.tensor.matmul(rr[:, i * B:i * B + NQ],
                                             w[:, tiles[i]], rhs,
                                             start=(first and i == 0),
                                             stop=(i == JQ and rr is not r1))
                    # DVE assembly:  T = A1 + A2 - AD  (RD already holds -AD)
                    #   y[0:128]   = A1[0:128]
                    #   y[128:240] = A1[128:240] + T[0:112]
                    #   y[240:256] = T[112:128]
                    #   y[256:368] = A2[0:112] + T[128:240] + Rg[0:112]
                    #   y[368:496] = A2[112:240] + Rg[112:240]
                    #   y[496:512] = Rg[240:256]
                    # DVE assembly (each op reads at most ONE PSUM operand,
                    # so A2 is staged into SBUF first):
                    #   T = A1 + A2 - AD  (cross, 16 blocks -> y blocks 8..23)
                    #   y[0:128]   = A1[0:128]
                    #   y[128:256] = A1[128:256] + T[0:128]
                    #   y[256:384] = A2[0:128] + T[128:256] + Rg[0:128]
                    #   y[384:512] = A2[128:256] + Rg[128:256]
                    add = mybir.AluOpType.add
                    s2 = spool.tile([C, NF], mybir.dt.float32)
                    nc.vector.tensor_copy(out=s2, in_=r2)
                    t1 = spool.tile([C, NF], mybir.dt.float32)
                    nc.vector.tensor_tensor(out=t1, in0=r1, in1=s2, op=add)
                    t2 = spool.tile([C, NF], mybir.dt.float32)
                    nc.vector.tensor_tensor(out=t2, in0=rd, in1=t1, op=add)
                    yd = yg[:, dd]
                    nc.vector.tensor_copy(out=yd[:, 0:NQ], in_=r1[:, 0:NQ])
                    nc.vector.tensor_tensor(out=yd[:, NQ:NF],
                                            in0=r1[:, NQ:NF],
                                            in1=t2[:, 0:NQ], op=add)
                    u = spool.tile([C, NQ], mybir.dt.float32)
                    nc.vector.tensor_tensor(out=u, in0=s2[:, 0:NQ],
                                            in1=t2[:, NQ:NF], op=add)
                    nc.vector.tensor_tensor(out=yd[:, NH:NH + NQ],
                                            in0=rg[:, 0:NQ], in1=u, op=add)
                    nc.vector.tensor_tensor(out=yd[:, NH + NQ:],
                                            in0=rg[:, NQ:NF],
                                            in1=s2[:, NQ:NF], op=add)
                eng_b.dma_start(out=yt[:, sl, :], in_=yg)

    nc.compile()
    _CACHE["nc"] = nc
    return nc


def _toep_tiles(gen, nlag):
    """PE-ready lhsT Toeplitz tiles from per-channel generators.

    gen: [DC, Tlen] e3m4, logically zero outside [0, Tlen).
    Returns tiles[c, d, k, a] = gen[d, 128k + a - c] for k in [0, nlag).
    """
    dc, tlen = gen.shape
    span = nlag * C
    gp = np.zeros((dc, 127 + span), dtype=F8)
    gp[:, 127:127 + min(tlen, span)] = gen[:, :span]
    base = gp[:, 127:]
    sv = np.lib.stride_tricks.as_strided(
        base,
        shape=(C, dc, span),
        strides=(-gp.strides[1], gp.strides[0], gp.strides[1]),
    )
    return np.ascontiguousarray(sv).reshape(C, dc, nlag, C)


def _prep_core_inputs(x, f, core):
    ds = slice(core * DC, (core + 1) * DC)
    xs = x[:, ds, :] * np.float32(1.0 / FSCALE)

    def to_dev(a, nblk):  # [B, DC, nblk*C] -> [C, DC, nblk*B] fp16
        v = a.reshape(B, DC, nblk, C).transpose(3, 1, 2, 0).reshape(C, DC, nblk * B)
        return np.ascontiguousarray(v).astype(F16)

    xt = to_dev(xs, J)
    zsum = xs[..., :L // 2] + xs[..., L // 2:]
    zst = to_dev(zsum, JH)
    xdiff = xs[..., L // 4:L // 2] - xs[..., :L // 4]
    xdt = to_dev(xdiff, JQ)

    fs = f[ds] * np.float32(FSCALE)
    q0 = fs[:, :L // 2].astype(F8)          # f0 quantized (2048)
    qg2 = (fs[:, L // 2:] - fs[:, :L // 2]).astype(F8)   # f1 - f0
    qad = (fs[:, :L // 4] - fs[:, L // 4:L // 2]).astype(F8)  # f00 - f01

    g1 = _toep_tiles(q0, JH)                      # lags 0..15 of f0
    a1b = _toep_tiles(q0[:, :L // 4], JQ + 1)[:, :, JQ - 1:]   # f00 lags 7,8
    a2f = _toep_tiles(q0[:, L // 4:], JQ + 1)
    a2b = np.concatenate([a2f[:, :, 0:1], a2f[:, :, JQ:]], axis=2)  # lags 0,8
    g2 = _toep_tiles(qg2, JH)
    ad = _toep_tiles(qad, JQ + 1)                 # lags 0..8 incl boundary
    ftiles = np.concatenate([g1, a1b, a2b, g2, ad], axis=2)
    assert ftiles.shape == (C, DC, NT, C)
    return {"xt": xt, "zs": zst, "xd": xdt, "ft": np.ascontiguousarray(ftiles)}


def _run(x, f, trace=False):
    from concourse.bass_utils import run_bass_kernel_spmd

    nc = _build_nc()
    in_maps = [_prep_core_inputs(x, f, i) for i in range(NCORES)]
    res = run_bass_kernel_spmd(
        nc, in_maps, core_ids=list(range(NCORES)), trace=trace
    )

    y = np.empty((B, D, L), dtype=np.float32)
    for i in range(NCORES):
        ytc = np.asarray(res.results[i]["yt"]).astype(np.float32)  # [C(a), DC, N]
        ys = ytc.reshape(C, DC, J, B).transpose(3, 1, 2, 0).reshape(B, DC, L)
        y[:, i * DC:(i + 1) * DC, :] = ys
    return y, res


def kernel(x, filter):
    x = np.asarray(x, dtype=np.float32)
    f = np.asarray(filter, dtype=np.float32)
    y, _ = _run(x, f, trace=False)
    return y


# revision 17
# speedup vs baseline: 1.1441x; 1.1441x over previous
"""FFT causal long-conv (H3/Hyena fftconv) as a blocked-Toeplitz matmul kernel
for 8 Trainium2 NeuronCores.

Math: y[b,d,l] = sum_{t<=l} filter[d,t] * x[b,d,l-t]  (causal conv, L taps).

Instead of an on-device FFT, the causal conv is computed directly as a
lower-block-triangular Toeplitz matmul: with 128-wide blocks (J=L/128 blocks),
y_i = sum_{k<=i} T_k @ x_{i-k} where T_k[a,c] = f[128k + a - c].  The T_k are
materialized host-side (bf16) as PE-ready lhsT tiles, so the device does only
dense [128,128]x[128,N] matmuls accumulating in fp32 PSUM — no transposes,
no twiddles.  MAC count is L^2/2 per (b,d) pair, which at L=4096 beats a
two-stage FFT factorization, and every DMA is a clean contiguous transfer.

Sharding: channels D=1024 split 128 per core (data-parallel over D, per the
independence of each channel's conv); all B=16 batches stay on-core so each
matmul gets the full N=512 free dim.

Weights are stored as fp8 E3M4 (filter pre-scaled by 64, x pre-scaled by 1/64
so no device-side dequant is needed): the PE streams fp8e3 lhsT at the same
1 col/cycle as fp16, but weight DMA bytes halve, taking the 16 SDMA engines
(~90% busy with fp16 weights) off the critical path.
"""

import numpy as np
import ml_dtypes


B, D, L = 16, 1024, 4096
NCORES = 8
DC = D // NCORES  # channels per core
C = 128           # time-block size == PE contraction dim
J = L // C        # 32 time blocks
N = J * B         # 512 = matmul free dim (j-block outer, batch inner)
GROUP = 4         # channels per DMA batch

F16 = np.float16
F8 = ml_dtypes.float8_e3m4
FSCALE = 64.0     # filter pre-scale into e3m4's sweet spot; x carries 1/64

_CACHE = {}


def _build_nc():
    if "nc" in _CACHE:
        return _CACHE["nc"]

    import concourse.bacc as bacc
    import concourse.tile as tile
    import concourse.mybir as mybir

    nc = bacc.Bacc("TRN2", target_bir_lowering=False, debug=False, num_devices=NCORES)

    # Layouts are chosen so every DMA has long contiguous per-partition runs:
    #   xt[c, d, n]    n = j*B + b         (input, time-within-block on partitions)
    #   ft[c, d, k, a] = f[d, 128k + a - c] (PE-ready lhsT Toeplitz tiles)
    #   yt[a, d, n]    n = i*B + b         (output)
    xt = nc.dram_tensor("xt", [C, DC, N], mybir.dt.float16, kind="ExternalInput")
    ft = nc.dram_tensor("ft", [C, DC, J, C], mybir.dt.float8e3, kind="ExternalInput")
    yt = nc.dram_tensor("yt", [C, DC, N], mybir.dt.float16, kind="ExternalOutput")

    with tile.TileContext(nc) as tc:
        with (
            tc.tile_pool(name="wpool", bufs=3) as wpool,
            tc.tile_pool(name="xpool", bufs=3) as xpool,
            tc.tile_pool(name="ypool", bufs=3) as ypool,
            tc.tile_pool(name="pspool", bufs=7, space="PSUM") as pspool,
            tc.tile_pool(name="warmps", bufs=1, space="PSUM") as warmps,
        ):
            # The PE otherwise idles ~12us waiting for the first weight DMA
            # and then pays the HAM half-clock ramp. Run throwaway matmuls on
            # a zeroed tile during that window so the array starts warm.
            wz = wpool.tile([C, N], mybir.dt.float16, tag="warmz", bufs=1)
            nc.vector.memset(wz, 0.0)
            wps = warmps.tile([C, N], mybir.dt.float32)
            # fp8 weights arrive ~25% sooner than the fp16 tuning point, so a
            # slightly shorter warm window suffices (still >3.4us for HAM).
            for _ in range(132):
                nc.tensor.matmul(wps[:, :64], wz[:, :C], wz[:, :64],
                                 start=True, stop=True)
            for g in range(DC // GROUP):
                sl = slice(g * GROUP, (g + 1) * GROUP)
                # Keep both HWDGE rings (SP + ACT) continuously busy: each
                # group's weight load is split half/half across the rings.
                # Everything stays off the slow gpsimd SWDGE path.
                eng_a = nc.sync if g % 2 == 0 else nc.scalar
                eng_b = nc.scalar if g % 2 == 0 else nc.sync
                xg = xpool.tile([C, GROUP, N], mybir.dt.float16)
                eng_b.dma_start(out=xg, in_=xt[:, sl, :])
                wt = wpool.tile([C, GROUP, J, C], mybir.dt.float8e3)
                h = 3 * GROUP // 4
                eng_a.dma_start(out=wt[:, :h], in_=ft[:, sl.start:sl.start + h, :, :])
                eng_b.dma_start(out=wt[:, h:], in_=ft[:, sl.start + h:sl.stop, :, :])
                yg = ypool.tile([C, GROUP, N], mybir.dt.float16)
                for dd in range(GROUP):
                    ps = pspool.tile([C, N], mybir.dt.float32)
                    for k in range(J):
                        ncols = (J - k) * B
                        nc.tensor.matmul(
                            ps[:, k * B:],
                            wt[:, dd, k, :],
                            xg[:, dd, :ncols],
                            start=(k == 0),
                            stop=(k == J - 1),
                        )
                    nc.vector.tensor_copy(out=yg[:, dd, :], in_=ps[:])
                eng_b.dma_start(out=yt[:, sl, :], in_=yg)

    nc.compile()
    _CACHE["nc"] = nc
    return nc


def _prep_core_inputs(x, f, core):
    ds = slice(core * DC, (core + 1) * DC)
    xs = x[:, ds, :].reshape(B, DC, J, C).transpose(3, 1, 2, 0).reshape(C, DC, N)
    xt = np.ascontiguousarray(xs * np.float32(1.0 / FSCALE)).astype(F16)

    # fpad[d, 127 + t] = f[d, t]; ft[c, d, m] = fpad[d, 127 + m - c]
    fpad = np.zeros((DC, 127 + L), dtype=F8)
    fpad[:, 127:] = (f[ds] * np.float32(FSCALE)).astype(F8)
    base = fpad[:, 127:]
    sv = np.lib.stride_tricks.as_strided(
        base,
        shape=(C, DC, L),
        strides=(-fpad.strides[1], fpad.strides[0], fpad.strides[1]),
    )
    ft = np.ascontiguousarray(sv).reshape(C, DC, J, C)
    return {"xt": xt, "ft": ft}


def _run(x, f, trace=False):
    from concourse.bass_utils import run_bass_kernel_spmd

    nc = _build_nc()
    in_maps = [_prep_core_inputs(x, f, i) for i in range(NCORES)]
    res = run_bass_kernel_spmd(
        nc, in_maps, core_ids=list(range(NCORES)), trace=trace
    )

    y = np.empty((B, D, L), dtype=np.float32)
    for i in range(NCORES):
        ytc = np.asarray(res.results[i]["yt"]).astype(np.float32)  # [C(a), DC, N]
        ys = ytc.reshape(C, DC, J, B).transpose(3, 1, 2, 0).reshape(B, DC, L)
        y[:, i * DC:(i + 1) * DC, :] = ys
    return y, res


def kernel(x, filter):
    x = np.asarray(x, dtype=np.float32)
    f = np.asarray(filter, dtype=np.float32)
    y, _ = _run(x, f, trace=False)
    return y



# revision 19
# speedup vs baseline: 1.1570x; 1.0113x over previous
"""FFT causal long-conv (H3/Hyena fftconv) as a blocked-Toeplitz matmul kernel
for 8 Trainium2 NeuronCores.

Math: y[b,d,l] = sum_{t<=l} filter[d,t] * x[b,d,l-t]  (causal conv, L taps).

Instead of an on-device FFT, the causal conv is computed directly as a
lower-block-triangular Toeplitz matmul: with 128-wide blocks (J=L/128 blocks),
y_i = sum_{k<=i} T_k @ x_{i-k} where T_k[a,c] = f[128k + a - c].  The T_k are
materialized host-side (bf16) as PE-ready lhsT tiles, so the device does only
dense [128,128]x[128,N] matmuls accumulating in fp32 PSUM — no transposes,
no twiddles.  MAC count is L^2/2 per (b,d) pair, which at L=4096 beats a
two-stage FFT factorization, and every DMA is a clean contiguous transfer.

Sharding: channels D=1024 split 128 per core (data-parallel over D, per the
independence of each channel's conv); all B=16 batches stay on-core so each
matmul gets the full N=512 free dim.

Weights are stored as fp8 E3M4 (filter pre-scaled by 64, x pre-scaled by 1/64
so no device-side dequant is needed): the PE streams fp8e3 lhsT at the same
1 col/cycle as fp16, but weight DMA bytes halve, taking the 16 SDMA engines
(~90% busy with fp16 weights) off the critical path.
"""

import numpy as np
import ml_dtypes


B, D, L = 16, 1024, 4096
NCORES = 8
DC = D // NCORES  # channels per core
C = 128           # time-block size == PE contraction dim
J = L // C        # 32 time blocks
N = J * B         # 512 = matmul free dim (j-block outer, batch inner)
GROUP = 4         # channels per DMA batch

F16 = np.float16
F8 = ml_dtypes.float8_e3m4
FSCALE = 64.0     # filter pre-scale into e3m4's sweet spot; x carries 1/64

_CACHE = {}


def _build_nc():
    if "nc" in _CACHE:
        return _CACHE["nc"]

    import concourse.bacc as bacc
    import concourse.tile as tile
    import concourse.mybir as mybir

    nc = bacc.Bacc("TRN2", target_bir_lowering=False, debug=False, num_devices=NCORES)

    # Layouts are chosen so every DMA has long contiguous per-partition runs:
    #   xt[c, d, n]    n = j*B + b         (input, time-within-block on partitions)
    #   ft[c, d, k, a] = f[d, 128k + a - c] (PE-ready lhsT Toeplitz tiles)
    #   yt[a, d, n]    n = i*B + b         (output)
    xt = nc.dram_tensor("xt", [C, DC, N], mybir.dt.float16, kind="ExternalInput")
    ft = nc.dram_tensor("ft", [C, DC, J, C], mybir.dt.float8e3, kind="ExternalInput")
    yt = nc.dram_tensor("yt", [C, DC, N], mybir.dt.float16, kind="ExternalOutput")

    with tile.TileContext(nc) as tc:
        with (
            tc.tile_pool(name="wpool", bufs=3) as wpool,
            tc.tile_pool(name="xpool", bufs=3) as xpool,
            tc.tile_pool(name="ypool", bufs=3) as ypool,
            tc.tile_pool(name="pspool", bufs=7, space="PSUM") as pspool,
            tc.tile_pool(name="warmps", bufs=1, space="PSUM") as warmps,
        ):
            # The PE otherwise idles ~12us waiting for the first weight DMA
            # and then pays the HAM half-clock ramp. Run throwaway matmuls on
            # a zeroed tile during that window so the array starts warm.
            wz = wpool.tile([C, N], mybir.dt.float16, tag="warmz", bufs=1)
            nc.vector.memset(wz, 0.0)
            wps = warmps.tile([C, N], mybir.dt.float32)
            # fp8 weights arrive ~25% sooner than the fp16 tuning point, so a
            # slightly shorter warm window suffices (still >3.4us for HAM).
            for _ in range(132):
                nc.tensor.matmul(wps[:, :64], wz[:, :C], wz[:, :64],
                                 start=True, stop=True)
            for g in range(DC // GROUP):
                sl = slice(g * GROUP, (g + 1) * GROUP)
                # Keep both HWDGE rings (SP + ACT) continuously busy: each
                # group's weight load is split half/half across the rings.
                # Everything stays off the slow gpsimd SWDGE path.
                eng_a = nc.sync if g % 2 == 0 else nc.scalar
                eng_b = nc.scalar if g % 2 == 0 else nc.sync
                xg = xpool.tile([C, GROUP, N], mybir.dt.float16)
                eng_b.dma_start(out=xg, in_=xt[:, sl, :])
                wt = wpool.tile([C, GROUP, J, C], mybir.dt.float8e3)
                h = 3 * GROUP // 4
                eng_a.dma_start(out=wt[:, :h], in_=ft[:, sl.start:sl.start + h, :, :])
                eng_b.dma_start(out=wt[:, h:], in_=ft[:, sl.start + h:sl.stop, :, :])
                yg = ypool.tile([C, GROUP, N], mybir.dt.float16)
                for dd in range(GROUP):
                    ps = pspool.tile([C, N], mybir.dt.float32)
                    for k in range(J):
                        ncols = (J - k) * B
                        nc.tensor.matmul(
                            ps[:, k * B:],
                            wt[:, dd, k, :],
                            xg[:, dd, :ncols],
                            start=(k == 0),
                            stop=(k == J - 1),
                        )
                    nc.vector.tensor_copy(out=yg[:, dd, :], in_=ps[:])
                eng_b.dma_start(out=yt[:, sl, :], in_=yg)

    nc.compile()
    _CACHE["nc"] = nc
    return nc


def _prep_core_inputs(x, f, core):
    ds = slice(core * DC, (core + 1) * DC)
    xs = x[:, ds, :].reshape(B, DC, J, C).transpose(3, 1, 2, 0).reshape(C, DC, N)
    xt = np.ascontiguousarray(xs * np.float32(1.0 / FSCALE)).astype(F16)

    # fpad[d, 127 + t] = f[d, t]; ft[c, d, m] = fpad[d, 127 + m - c]
    fpad = np.zeros((DC, 127 + L), dtype=F8)
    fpad[:, 127:] = (f[ds] * np.float32(FSCALE)).astype(F8)
    base = fpad[:, 127:]
    sv = np.lib.stride_tricks.as_strided(
        base,
        shape=(C, DC, L),
        strides=(-fpad.strides[1], fpad.strides[0], fpad.strides[1]),
    )
    ft = np.ascontiguousarray(sv).reshape(C, DC, J, C)
    return {"xt": xt, "ft": ft}


def _run(x, f, trace=False):
    from concourse.bass_utils import run_bass_kernel_spmd

    nc = _build_nc()
    in_maps = [_prep_core_inputs(x, f, i) for i in range(NCORES)]
    res = run_bass_kernel_spmd(
        nc, in_maps, core_ids=list(range(NCORES)), trace=trace
    )

    y = np.empty((B, D, L), dtype=np.float32)
    for i in range(NCORES):
        ytc = np.asarray(res.results[i]["yt"]).astype(np.float32)  # [C(a), DC, N]
        ys = ytc.reshape(C, DC, J, B).transpose(3, 1, 2, 0).reshape(B, DC, L)
        y[:, i * DC:(i + 1) * DC, :] = ys
    return y, res


def kernel(x, filter):
    x = np.asarray(x, dtype=np.float32)
    f = np.asarray(filter, dtype=np.float32)
    y, _ = _run(x, f, trace=False)
    return y

